# revision 12
# baseline (speedup 1.0000x reference)
"""Trainium2 Bass kernel for FlashMultiHeadAttention (B=2, L=2048, D=1024, H=16, Dh=64).

Sharding: 8 cores = 2 (batch) x 4 (head groups of 4 heads).
Per core (batch b, head group hg, 4 heads):
  - Q^T, K^T projections ([256, L], head dim on partitions) with RoPE applied
    during PSUM evacuation: ACT copies the PSUM tile to SBUF bf16, a DMA
    partition-swap produces the rotate_half copy, and three 2x-mode bf16 DVE
    ops (mul cos, mul signed-sin, add) finish the rotation. U, V projected in
    natural [L, 256] layout with biases and the action gate folded in via
    augmented contraction rows.
  - Scores computed transposed (S^T[k, q]) so the learned time-delta bias is a
    per-partition bias. The exp pass is SPLIT between the Scalar engine
    (fused scale+bias+exp activation) and the Vector engine, which computes an
    approximate exp in ONE tensor_scalar op via a float-mantissa trick:
        u = fp32( score * (128*log2e*scale) + (2^23 + 127*128 - 7 + bias') )
    rounds the bf16 bit pattern of exp(.) into the low half-word of u; a
    stride-2 bf16 view of the fp32 tile feeds the PV matmul directly.
  - PV carries an extra all-ones column producing softmax denominators.
    Normalization: approx-reciprocal straight from PSUM, SWDGE
    partition-broadcast of 1/den, and one fused PSUMxSBUF multiply that both
    evacuates and normalizes the numerators.
  - Row-sliced output projection -> partial outT [1024, 2048] bf16.
Host sums the 4 head-group partials per batch and adds bo.

PSUM: two tags of 2 x [128,1024] fp32 slots (8 banks total) shared by all
phases. The attention kt-loop is software-pipelined (QK one step ahead of PV)
so the PE never waits for the exp pass.
"""

import sys

if "/opt/trn_rl_repo" not in sys.path:
    sys.path.insert(0, "/opt/trn_rl_repo")

import numpy as np
import ml_dtypes

BF16 = ml_dtypes.bfloat16

B = 2
L = 2048
D = 1024
H = 16
DH = 64
NG = 256          # head dims per group (4 heads)
DPAD = 1152       # padded contraction rows (9 * 128)
NCORES = 8
SCALE = DH ** -0.5
EXP_A = 184.66496523378731          # 128 / ln(2)
EXP_B = 8388608.0 + 127 * 128 - 7   # 2^23 + bf16 exponent bias - correction
ACT_FRAC = 16                       # of 16 exp tiles go to ACT; rest DVE


def build_bass(with_mask: bool):
    """Build the single-core SPMD Bass program (same program on all 8 cores)."""
    import concourse.mybir as mybir
    from concourse import bacc
    from concourse.tile import TileContext

    f32 = mybir.dt.float32
    bf16 = mybir.dt.bfloat16
    EXP = mybir.ActivationFunctionType.Exp
    TANH = mybir.ActivationFunctionType.Tanh

    nc = bacc.Bacc(None, target_bir_lowering=False)

    xq = nc.dram_tensor("xq", [DPAD, L], bf16, kind="ExternalInput")
    xk = nc.dram_tensor("xk", [DPAD, L], bf16, kind="ExternalInput")
    xv = nc.dram_tensor("xv", [DPAD, L], bf16, kind="ExternalInput")
    wq = nc.dram_tensor("wq", [DPAD, NG], bf16, kind="ExternalInput")
    wu = nc.dram_tensor("wu", [DPAD, NG], bf16, kind="ExternalInput")
    wk = nc.dram_tensor("wk", [DPAD, NG], bf16, kind="ExternalInput")
    wv = nc.dram_tensor("wv", [DPAD, NG], bf16, kind="ExternalInput")
    wo = nc.dram_tensor("wo", [NG, D], bf16, kind="ExternalInput")
    cb = nc.dram_tensor("cb", [128, 64], f32, kind="ExternalInput")
    ce = nc.dram_tensor("ce", [128, 64], f32, kind="ExternalInput")
    cs = nc.dram_tensor("cs", [128, L], bf16, kind="ExternalInput")
    sn = nc.dram_tensor("sn", [128, L], bf16, kind="ExternalInput")
    mk = None
    if with_mask:
        mk = nc.dram_tensor("mk", [L, L], f32, kind="ExternalInput")
    outT = nc.dram_tensor("outT", [D, L], bf16, kind="ExternalOutput")

    dma = nc.default_dma_engine

    with TileContext(nc) as tc:
        with tc.tile_pool(name="persist", bufs=1) as persist, \
             tc.tile_pool(name="xkp", bufs=1) as xkpool, \
             tc.tile_pool(name="ps", bufs=1, space="PSUM") as ps, \
             tc.tile_pool(name="ev", bufs=2) as ev, \
             tc.tile_pool(name="ptp", bufs=3) as ptpool, \
             tc.tile_pool(name="nrm", bufs=2) as nrmpool, \
             tc.tile_pool(name="drm", bufs=2, space="DRAM") as drmpool, \
             tc.tile_pool(name="mkp", bufs=4) as mkpool:
            qT = [persist.tile([128, L], bf16, name=f"qT{n}") for n in range(2)]
            kT = [persist.tile([128, L], bf16, name=f"kT{n}") for n in range(2)]
            vg = persist.tile([128, 16 * 260], bf16, name="vg")
            oT = [persist.tile([128, L], bf16, name=f"oT{n}") for n in range(2)]
            sig = [persist.tile([128, 1024], bf16, name=f"sig{qc}") for qc in range(4)]
            cbS = persist.tile([128, 64], f32, name="cbS")
            ceS = persist.tile([128, 64], f32, name="ceS")
            csS = persist.tile([128, L], bf16, name="csS")
            snS = persist.tile([128, L], bf16, name="snS")
            woS = [persist.tile([128, D], bf16, name=f"woS{n2}") for n2 in range(2)]
            wqS = persist.tile([128, 9 * NG], bf16, name="wqS")
            wuS = persist.tile([128, 9 * NG], bf16, name="wuS")
            wkS = persist.tile([128, 9 * NG], bf16, name="wkS")
            wvS = persist.tile([128, 9 * NG], bf16, name="wvS")
            for wt_sb, wt_dr in ((wqS, wq), (wuS, wu)):
                dma.dma_start(out=wt_sb.rearrange("p (c n) -> p c n", n=NG),
                              in_=wt_dr.rearrange("(c p) n -> p c n", p=128))
            wqS3 = wqS.rearrange("p (c n) -> p c n", n=NG)
            wuS3 = wuS.rearrange("p (c n) -> p c n", n=NG)
            wkS3 = wkS.rearrange("p (c n) -> p c n", n=NG)
            wvS3 = wvS.rearrange("p (c n) -> p c n", n=NG)

            vg4 = vg.rearrange("p (t h e) -> p t h e", h=4, e=65)
            nc.vector.memset(vg4[:, :, :, 64:65], 1.0)

            xkS = xkpool.tile([128, 9 * L], bf16, name="xkS")
            xkS3 = xkS.rearrange("p (c q) -> p c q", q=L)

            def slot(tag, name, p=128):
                return ps.tile([p, 1024], f32, tag=tag, bufs=2, name=name)

            def rope_evac(pps, dest, s):
                """dest[:, s] = pps*cos + rotate_half(pps)*signed_sin.

                pps: [128, 512] PSUM fp32. ACT evacuates to bf16, a DMA
                partition-swap (+-32 within each 64-block) makes the rotated
                copy, three bf16 DVE ops finish.
                """
                e = ev.tile([128, 512], bf16, tag="e", bufs=3, name="e")
                r = ev.tile([128, 512], bf16, tag="r", bufs=3, name="r")
                nc.scalar.copy(e, pps)
                for blk in (0, 64):
                    nc.scalar.dma_start(out=r[blk:blk + 32],
                                        in_=e[blk + 32:blk + 64])
                    nc.scalar.dma_start(out=r[blk + 32:blk + 64],
                                        in_=e[blk:blk + 32])
                tc_ = ev.tile([128, 512], bf16, tag="tc", name="tc_")
                tr_ = ev.tile([128, 512], bf16, tag="tr", name="tr_")
                nc.vector.tensor_mul(tc_, e, csS[:, s])
                nc.vector.tensor_mul(tr_, r, snS[:, s])
                nc.vector.tensor_add(dest[:, s], tc_, tr_)

            # ---- QU: Q^T (+RoPE) then U + sigmoid, per q-chunk ----
            with tc.tile_pool(name="xqp", bufs=1) as xqpool:
                xqS = xqpool.tile([128, 9 * L], bf16, name="xqS")
                xqS3 = xqS.rearrange("p (c q) -> p c q", q=L)
                for qc in range(4):
                    s = slice(qc * 512, (qc + 1) * 512)
                    for d in range(9):
                        dma.dma_start(out=xqS3[:, d, s],
                                      in_=xq[d * 128:(d + 1) * 128, s])
                # tables + wk/wv on the second HWDGE ring so they arrive in
                # parallel with the xq stream
                nc.scalar.dma_start(out=csS, in_=cs[:, :])
                nc.scalar.dma_start(out=snS, in_=sn[:, :])
                nc.scalar.dma_start(out=cbS, in_=cb[:, :])
                nc.scalar.dma_start(out=ceS, in_=ce[:, :])
                for n2 in range(2):
                    nc.scalar.dma_start(out=woS[n2], in_=wo[n2 * 128:(n2 + 1) * 128, :])
                for wt_sb, wt_dr in ((wkS, wk), (wvS, wv)):
                    dma.dma_start(out=wt_sb.rearrange("p (c n) -> p c n", n=NG),
                                  in_=wt_dr.rearrange("(c p) n -> p c n", p=128))
                for qc in range(4):
                    s = slice(qc * 512, (qc + 1) * 512)
                    for d in range(9):
                        dma.dma_start(out=xkS3[:, d, s],
                                      in_=xk[d * 128:(d + 1) * 128, s])

                for qc in range(4):
                    s = slice(qc * 512, (qc + 1) * 512)
                    qps = slot("st", "qps")
                    for d in range(9):
                        xt = xqS3[:, d, s]
                        for n in range(2):
                            nc.tensor.matmul(qps[:, n * 512:(n + 1) * 512],
                                             lhsT=wqS3[:, d, n * 128:(n + 1) * 128],
                                             rhs=xt, start=(d == 0), stop=(d == 8))
                    for n in range(2):
                        rope_evac(qps[:, n * 512:(n + 1) * 512], qT[n], s)
                    ups = slot("pv", "ups")
                    for i in range(4):
                        for d in range(9):
                            xt = xqS3[:, d, s]
                            nc.tensor.matmul(ups[:, i * 256:(i + 1) * 256],
                                             lhsT=xt[:, i * 128:(i + 1) * 128],
                                             rhs=wuS3[:, d, :],
                                             start=(d == 0), stop=(d == 8))
                    # sigmoid(u) = 0.5*tanh(0.5*u) + 0.5
                    eu = ev.tile([128, 1024], f32, tag="eu", bufs=1, name="eu")
                    nc.scalar.activation(out=eu, in_=ups, func=TANH, scale=0.5)
                    nc.vector.tensor_scalar(sig[qc], eu, 0.5, 0.5,
                                            mybir.AluOpType.mult,
                                            mybir.AluOpType.add)

            # ---- KV: K^T (+RoPE) then V + gating, per q-chunk ----
            with tc.tile_pool(name="xvp", bufs=1) as xvpool:
                xvS = xvpool.tile([128, 9 * L], bf16, name="xvS")
                xvS3 = xvS.rearrange("p (c q) -> p c q", q=L)
                for qc in range(4):
                    s = slice(qc * 512, (qc + 1) * 512)
                    for d in range(9):
                        dma.dma_start(out=xvS3[:, d, s],
                                      in_=xv[d * 128:(d + 1) * 128, s])
                for qc in range(4):
                    s = slice(qc * 512, (qc + 1) * 512)
                    kps = slot("st", "kps")
                    for d in range(9):
                        xtk = xkS3[:, d, s]
                        for n in range(2):
                            nc.tensor.matmul(kps[:, n * 512:(n + 1) * 512],
                                             lhsT=wkS3[:, d, n * 128:(n + 1) * 128],
                                             rhs=xtk, start=(d == 0), stop=(d == 8))
                    for n in range(2):
                        rope_evac(kps[:, n * 512:(n + 1) * 512], kT[n], s)
                    vps = slot("pv", "vps")
                    for i in range(4):
                        for d in range(9):
                            xtv = xvS3[:, d, s]
                            nc.tensor.matmul(vps[:, i * 256:(i + 1) * 256],
                                             lhsT=xtv[:, i * 128:(i + 1) * 128],
                                             rhs=wvS3[:, d, :],
                                             start=(d == 0), stop=(d == 8))
                    vcp = ev.tile([128, 1024], bf16, tag="vc", name="vcp")
                    nc.scalar.copy(vcp, vps)
                    for i in range(4):
                        kt_g = qc * 4 + i
                        vsrc = vcp[:, i * 256:(i + 1) * 256].rearrange(
                            "p (h e) -> p h e", e=64)
                        ssrc = sig[qc][:, i * 256:(i + 1) * 256].rearrange(
                            "p (h e) -> p h e", e=64)
                        nc.vector.tensor_mul(vg4[:, kt_g, :, 0:64], vsrc, ssrc)

            # ---- Attention (software-pipelined: QK one kt ahead of PV) ----
            for h in range(4):
                n = h // 2
                r0 = (h % 2) * 64
                pvt = [slot("pv", f"pvt{hq}", p=65) for hq in range(2)]
                pts = [None, None]          # pt rhs views for PV of kt-1

                def emit_pv(kt):
                    for hq in range(2):
                        for s2 in range(2):
                            nc.tensor.matmul(
                                pvt[hq][:, s2 * 512:(s2 + 1) * 512],
                                lhsT=vg[:, kt * 260 + h * 65:kt * 260 + h * 65 + 65],
                                rhs=pts[hq][:, s2 * 512:(s2 + 1) * 512],
                                start=(kt == 0), stop=(kt == 15))

                for kt in range(16):
                    col = kt * 4 + h
                    new_pts = []
                    for hq in range(2):
                        st = slot("st", "st")
                        for s2 in range(2):
                            q0 = hq * 1024 + s2 * 512
                            nc.tensor.matmul(
                                st[:, s2 * 512:(s2 + 1) * 512],
                                lhsT=kT[n][r0:r0 + 64, kt * 128:(kt + 1) * 128],
                                rhs=qT[n][r0:r0 + 64, q0:q0 + 512],
                                start=True, stop=True)
                        if with_mask:
                            mt = mkpool.tile([128, 1024], f32, tag="mt", name="mt")
                            dma.dma_start(
                                out=mt,
                                in_=mk[kt * 128:(kt + 1) * 128,
                                       hq * 1024:(hq + 1) * 1024])
                            nc.vector.tensor_add(st, st, mt)
                        i16 = ((h * 16 + kt) * 2 + hq) % 16
                        if (i16 * ACT_FRAC // 16) != ((i16 + 1) * ACT_FRAC // 16):
                            pt = ptpool.tile([128, 1024], bf16, tag="pt",
                                             bufs=3, name="pt")
                            nc.scalar.activation(out=pt, in_=st, func=EXP,
                                                 scale=SCALE,
                                                 bias=cbS[:, col:col + 1])
                            new_pts.append(pt)
                        else:
                            pu = ptpool.tile([128, 1024], f32, tag="pu",
                                             bufs=3, name="pu")
                            nc.vector.tensor_scalar(pu, st,
                                                    EXP_A * SCALE,
                                                    ceS[:, col:col + 1],
                                                    mybir.AluOpType.mult,
                                                    mybir.AluOpType.add)
                            new_pts.append(
                                pu.bitcast(mybir.dt.bfloat16).rearrange(
                                    "p (q t) -> p q t", t=2)[:, :, 0])
                    if kt > 0:
                        emit_pv(kt - 1)
                    pts = new_pts
                emit_pv(15)

                # normalize: ACT-evacuate both denominators, DRAM-reshape to
                # [128,16], exact DVE reciprocal, reshape back to partition 0,
                # SWDGE partition broadcast, fused evacuate+normalize multiply.
                dsb = nrmpool.tile([65, 2048], f32, tag="ds", bufs=1, name="dsb")
                for hq in range(2):
                    nc.scalar.copy(dsb[64:65, hq * 1024:(hq + 1) * 1024],
                                   pvt[hq][64:65, :])
                drv = drmpool.tile([2048], f32, tag="dv", name="drv")
                dma.dma_start(out=drv[:], in_=dsb[64:65, :])
                dcol = nrmpool.tile([128, 16], f32, tag="dc", bufs=1, name="dcol")
                dma.dma_start(out=dcol,
                              in_=drv.rearrange("(p c) -> p c", p=128))
                rcol = nrmpool.tile([128, 16], f32, tag="rc", bufs=1, name="rcol")
                nc.vector.reciprocal(out=rcol, in_=dcol)
                dr2 = drmpool.tile([2048], f32, tag="d2", name="dr2")
                dma.dma_start(out=dr2.rearrange("(p c) -> p c", p=128),
                              in_=rcol)
                rrow = nrmpool.tile([1, 2048], f32, tag="rr", bufs=1, name="rrow")
                dma.dma_start(out=rrow, in_=dr2[:])
                nms = []
                for hq in range(2):
                    nm = nrmpool.tile([64, 1024], bf16, tag="nm", bufs=2,
                                      name="nm")
                    if hq == 0:
                        nc.scalar.copy(nm, pvt[hq][0:64, :])
                    else:
                        nc.vector.tensor_copy(out=nm, in_=pvt[hq][0:64, :])
                    nms.append(nm)
                for hq in range(2):
                    ib = nrmpool.tile([64, 1024], f32, tag="ib", name="ib")
                    nc.gpsimd.partition_broadcast(
                        ib, rrow[0:1, hq * 1024:(hq + 1) * 1024])
                    nc.vector.tensor_mul(
                        oT[n][r0:r0 + 64, hq * 1024:(hq + 1) * 1024],
                        nms[hq], ib)

            # keep the PE clock warm across the normalize tail of head 3
            wt_ = slot("st", "warm")
            for j_ in range(24):
                nc.tensor.matmul(wt_[:, 0:512], lhsT=woS[0][:, 0:128],
                                 rhs=woS[0][:, 0:512],
                                 start=(j_ == 0), stop=(j_ == 23))

            # ---- Out-projection ----
            with tc.tile_pool(name="otp", bufs=2) as otpool:
                for mt_i in range(8):
                    ot = otpool.tile([128, L], bf16, tag="ot", name="ot")
                    ops = [slot("st", "op0"), slot("pv", "op1")]
                    for n2 in range(2):
                        for qc in range(4):
                            nc.tensor.matmul(
                                ops[qc // 2][:, (qc % 2) * 512:(qc % 2) * 512 + 512],
                                lhsT=woS[n2][:, mt_i * 128:(mt_i + 1) * 128],
                                rhs=oT[n2][:, qc * 512:(qc + 1) * 512],
                                start=(n2 == 0), stop=(n2 == 1))
                    nc.scalar.copy(ot[:, 0:1024], ops[0])
                    nc.vector.tensor_copy(out=ot[:, 1024:2048], in_=ops[1])
                    dma.dma_start(out=outT[mt_i * 128:(mt_i + 1) * 128, :], in_=ot)

    nc.finalize()
    return nc


def prep_inputs(query, key, value, attn_mask, action_ids, time_deltas,
                Wq, bq, Wk, bk, Wv, bv, Wu, bu, Wo, bo,
                action_emb, Wap, bap, td_emb, td_gate):
    """Host-side sharding: build the 8 per-core input maps."""
    query = np.asarray(query, np.float32)
    key = np.asarray(key, np.float32)
    value = np.asarray(value, np.float32)
    attn_mask = np.asarray(attn_mask)
    action_ids = np.asarray(action_ids)
    time_deltas = np.asarray(time_deltas)

    sig_gate = 1.0 / (1.0 + np.exp(-np.float64(td_gate)))
    with_mask = not bool(attn_mask.all())

    xq_b, xk_b, xv_b, cb_b, mk_b = [], [], [], [], []
    for b in range(B):
        ae = np.asarray(action_emb, np.float32)[action_ids[b]]      # [L, 16]
        xqa = np.zeros((DPAD, L), BF16)
        xqa[:D] = query[b].T.astype(BF16)
        xqa[D:D + 16] = ae.T.astype(BF16)
        xqa[D + 16] = BF16(1.0)
        xq_b.append(xqa)
        xka = np.zeros((DPAD, L), BF16)
        xka[:D] = key[b].T.astype(BF16)
        xka[D] = BF16(1.0)
        xk_b.append(xka)
        xva = np.zeros((DPAD, L), BF16)
        xva[:D] = value[b].T.astype(BF16)
        xva[D] = BF16(1.0)
        xv_b.append(xva)
        tdc = np.clip(time_deltas[b].astype(np.int64), 0, td_emb.shape[0] - 1)
        cb_b.append((sig_gate * np.asarray(td_emb, np.float64)[tdc]))   # [L, H]
        if with_mask:
            m = np.where(attn_mask[b], np.float32(0.0), np.float32(-1e9))
            mk_b.append(np.ascontiguousarray(m.T))                  # [k, q]

    wq_a = np.zeros((DPAD, D), np.float32)
    wq_a[:D] = Wq
    wq_a[D + 16] = bq
    wu_a = np.zeros((DPAD, D), np.float32)
    wu_a[:D] = Wu
    wu_a[D:D + 16] = Wap
    wu_a[D + 16] = np.asarray(bu) + np.asarray(bap)
    wk_a = np.zeros((DPAD, D), np.float32)
    wk_a[:D] = Wk
    wk_a[D] = bk
    wv_a = np.zeros((DPAD, D), np.float32)
    wv_a[:D] = Wv
    wv_a[D] = bv

    # RoPE tables in [dh, pos] orientation, duplicated for the 2-head packing.
    # sin table carries the rotate_half sign: rows d<32 of each 64-block hold
    # -sin (they multiply q[d+32]), rows d>=32 hold +sin (multiply q[d-32]).
    inv_freq = 1.0 / (10000.0 ** (np.arange(0, DH, 2, dtype=np.float64) / DH))
    pos = np.arange(L, dtype=np.float64)
    freqs = pos[None, :] * inv_freq[:, None]            # [32, L]
    cos_t = np.repeat(np.cos(freqs), 2, axis=0)[:DH]    # [64, L]
    sin_t = np.repeat(np.sin(freqs), 2, axis=0)[:DH]
    ss_t = sin_t.copy()
    ss_t[0:32] = -ss_t[0:32]
    cs_t = np.ascontiguousarray(np.concatenate([cos_t, cos_t], 0)).astype(BF16)
    sn_t = np.ascontiguousarray(np.concatenate([ss_t, ss_t], 0)).astype(BF16)

    in_maps = []
    for c in range(NCORES):
        b, hg = c // 4, c % 4
        csl = slice(hg * NG, (hg + 1) * NG)
        cbc = cb_b[b][:, hg * 4:(hg + 1) * 4]                       # [L, 4]
        cbc = cbc.reshape(16, 128, 4).transpose(1, 0, 2).reshape(128, 64)
        m = {
            "xq": xq_b[b], "xk": xk_b[b], "xv": xv_b[b],
            "wq": wq_a[:, csl].astype(BF16), "wu": wu_a[:, csl].astype(BF16),
            "wk": wk_a[:, csl].astype(BF16), "wv": wv_a[:, csl].astype(BF16),
            "wo": np.asarray(Wo, np.float32)[csl, :].astype(BF16),
            "cb": np.ascontiguousarray(cbc).astype(np.float32),
            "ce": np.ascontiguousarray(EXP_B + EXP_A * cbc).astype(np.float32),
            "cs": cs_t, "sn": sn_t,
        }
        if with_mask:
            m["mk"] = mk_b[b]
        in_maps.append(m)
    return in_maps, with_mask


def gather_output(results, bo):
    """Sum head-group partials per batch, transpose, add bo."""
    out = np.empty((B, L, D), np.float32)
    for b in range(B):
        acc = results[b * 4]["outT"].astype(np.float32)
        for g in range(1, 4):
            acc = acc + results[b * 4 + g]["outT"].astype(np.float32)
        out[b] = acc.T + np.asarray(bo, np.float32)
    return out


def kernel(**inputs):
    from concourse.bass_utils import run_bass_kernel_spmd

    in_maps, with_mask = prep_inputs(**inputs)
    nc = build_bass(with_mask)
    res = run_bass_kernel_spmd(nc, in_maps, core_ids=list(range(NCORES)))
    return gather_output(res.results, inputs["bo"])


# revision 13
# speedup vs baseline: 1.2649x; 1.2649x over previous
"""Trainium2 Bass kernel for FlashMultiHeadAttention (B=2, L=2048, D=1024, H=16, Dh=64).

Sharding: 8 cores = 2 (batch) x 4 (head groups of 4 heads).
Per core (batch b, head group hg, 4 heads):
  - Q^T, K^T projections ([256, L], head dim on partitions) with RoPE applied
    during PSUM evacuation via partition-shifted DVE multiplies against
    cos / signed-sin tables; U, V projected in natural [L, 256] layout with
    biases and the action gate folded in via augmented contraction rows.
  - Scores computed transposed (S^T[k, q]) so the learned time-delta bias is a
    per-partition ACT bias fused into the exp instruction (scale+bias+exp+cast
    in one op). P^T feeds PV directly - no PE transposes anywhere.
  - PV carries an extra all-ones column producing softmax denominators;
    normalization uses a batched reciprocal + DRAM-broadcast of 1/r.
  - Row-sliced output projection -> partial outT [1024, 2048] fp32.
Host sums the 4 head-group partials per batch and adds bo.

A single PSUM pool (4x 1-bank "sm" slots + 2x 2-bank "st" slots) is shared by
every phase so there are no pool-release barriers; phases overlap by dataflow
and the PE never idles long enough to drop the HAM clock.
"""

import sys

if "/opt/trn_rl_repo" not in sys.path:
    sys.path.insert(0, "/opt/trn_rl_repo")

import numpy as np
import ml_dtypes

BF16 = ml_dtypes.bfloat16

B = 2
L = 2048
D = 1024
H = 16
DH = 64
NG = 256          # head dims per group (4 heads)
DPAD = 1152       # padded contraction rows (9 * 128)
NCORES = 8
SCALE = DH ** -0.5


def build_bass(with_mask: bool):
    """Build the single-core SPMD Bass program (same program on all 8 cores)."""
    import concourse.mybir as mybir
    from concourse import bacc
    from concourse.tile import TileContext

    f32 = mybir.dt.float32
    bf16 = mybir.dt.bfloat16
    EXP = mybir.ActivationFunctionType.Exp
    TANH = mybir.ActivationFunctionType.Tanh

    nc = bacc.Bacc(None, target_bir_lowering=False)

    xq = nc.dram_tensor("xq", [DPAD, L], bf16, kind="ExternalInput")
    xk = nc.dram_tensor("xk", [DPAD, L], bf16, kind="ExternalInput")
    xv = nc.dram_tensor("xv", [DPAD, L], bf16, kind="ExternalInput")
    wq = nc.dram_tensor("wq", [DPAD, NG], bf16, kind="ExternalInput")
    wu = nc.dram_tensor("wu", [DPAD, NG], bf16, kind="ExternalInput")
    wk = nc.dram_tensor("wk", [DPAD, NG], bf16, kind="ExternalInput")
    wv = nc.dram_tensor("wv", [DPAD, NG], bf16, kind="ExternalInput")
    wo = nc.dram_tensor("wo", [NG, D], bf16, kind="ExternalInput")
    cb = nc.dram_tensor("cb", [128, 64], f32, kind="ExternalInput")
    cs = nc.dram_tensor("cs", [128, L], f32, kind="ExternalInput")
    sn = nc.dram_tensor("sn", [128, L], f32, kind="ExternalInput")
    mk = None
    if with_mask:
        mk = nc.dram_tensor("mk", [L, L], f32, kind="ExternalInput")
    outT = nc.dram_tensor("outT", [D, L], f32, kind="ExternalOutput")

    dma = nc.default_dma_engine

    def rope_evac(pps, dest, csS, snS, s, pool):
        """dest[:, s] (bf16) = pps*cos + rotate_half(pps)*signed_sin."""
        tc_ = pool.tile([128, 512], f32, tag="tc", name="tc_")
        tr_ = pool.tile([128, 512], f32, tag="tr", name="tr_")
        nc.vector.tensor_mul(tc_, pps, csS[:, s])
        for blk in (0, 64):
            nc.vector.tensor_mul(tr_[blk:blk + 32],
                                 pps[blk + 32:blk + 64, :], snS[blk:blk + 32, s])
            nc.vector.tensor_mul(tr_[blk + 32:blk + 64],
                                 pps[blk:blk + 32, :], snS[blk + 32:blk + 64, s])
        nc.vector.tensor_add(dest[:, s], tc_, tr_)

    with TileContext(nc) as tc:
        with tc.tile_pool(name="persist", bufs=1) as persist, \
             tc.tile_pool(name="xkp", bufs=1) as xkpool, \
             tc.tile_pool(name="ps", bufs=1, space="PSUM") as ps, \
             tc.tile_pool(name="ev", bufs=2) as ev, \
             tc.tile_pool(name="ptp", bufs=6) as ptpool, \
             tc.tile_pool(name="nrm", bufs=2) as nrmpool, \
             tc.tile_pool(name="drm", bufs=2, space="DRAM") as drmpool, \
             tc.tile_pool(name="mkp", bufs=4) as mkpool:
            qT = [persist.tile([128, L], bf16, name=f"qT{n}") for n in range(2)]
            kT = [persist.tile([128, L], bf16, name=f"kT{n}") for n in range(2)]
            vg = persist.tile([128, 16 * 260], bf16, name="vg")
            oT = [persist.tile([128, L], bf16, name=f"oT{n}") for n in range(2)]
            sig = [persist.tile([128, 1024], bf16, name=f"sig{qc}") for qc in range(4)]
            cbS = persist.tile([128, 64], f32, name="cbS")
            csS = persist.tile([128, L], f32, name="csS")
            snS = persist.tile([128, L], f32, name="snS")
            woS = [persist.tile([128, D], bf16, name=f"woS{n2}") for n2 in range(2)]
            wqS = persist.tile([128, 9 * NG], bf16, name="wqS")
            wuS = persist.tile([128, 9 * NG], bf16, name="wuS")
            wkS = persist.tile([128, 9 * NG], bf16, name="wkS")
            wvS = persist.tile([128, 9 * NG], bf16, name="wvS")
            for wt_sb, wt_dr in ((wqS, wq), (wuS, wu)):
                dma.dma_start(out=wt_sb.rearrange("p (c n) -> p c n", n=NG),
                              in_=wt_dr.rearrange("(c p) n -> p c n", p=128))
            wqS3 = wqS.rearrange("p (c n) -> p c n", n=NG)
            wuS3 = wuS.rearrange("p (c n) -> p c n", n=NG)
            wkS3 = wkS.rearrange("p (c n) -> p c n", n=NG)
            wvS3 = wvS.rearrange("p (c n) -> p c n", n=NG)

            vg4 = vg.rearrange("p (t h e) -> p t h e", h=4, e=65)
            nc.vector.memset(vg4[:, :, :, 64:65], 1.0)

            xkS = xkpool.tile([128, 9 * L], bf16, name="xkS")
            xkS3 = xkS.rearrange("p (c q) -> p c q", q=L)

            def sm_tile(name):
                return ps.tile([128, 512], f32, tag="sm", bufs=4, name=name)

            # ---- QU: Q^T (+RoPE) then U + sigmoid, per q-chunk ----
            with tc.tile_pool(name="xqp", bufs=1) as xqpool:
                xqS = xqpool.tile([128, 9 * L], bf16, name="xqS")
                xqS3 = xqS.rearrange("p (c q) -> p c q", q=L)
                for qc in range(4):
                    s = slice(qc * 512, (qc + 1) * 512)
                    for d in range(9):
                        dma.dma_start(out=xqS3[:, d, s],
                                      in_=xq[d * 128:(d + 1) * 128, s])
                # cos/sin/bias/wo on the second HWDGE ring (ACT engine) so
                # they arrive in parallel with the xq stream
                nc.scalar.dma_start(out=csS, in_=cs[:, :])
                nc.scalar.dma_start(out=snS, in_=sn[:, :])
                nc.scalar.dma_start(out=cbS, in_=cb[:, :])
                for n2 in range(2):
                    nc.scalar.dma_start(out=woS[n2], in_=wo[n2 * 128:(n2 + 1) * 128, :])
                for wt_sb, wt_dr in ((wkS, wk), (wvS, wv)):
                    dma.dma_start(out=wt_sb.rearrange("p (c n) -> p c n", n=NG),
                                  in_=wt_dr.rearrange("(c p) n -> p c n", p=128))
                for qc in range(4):
                    s = slice(qc * 512, (qc + 1) * 512)
                    for d in range(9):
                        dma.dma_start(out=xkS3[:, d, s],
                                      in_=xk[d * 128:(d + 1) * 128, s])

                for qc in range(4):
                    s = slice(qc * 512, (qc + 1) * 512)
                    qps = [sm_tile(f"qps{n}") for n in range(2)]
                    for d in range(9):
                        xt = xqS3[:, d, s]
                        for n in range(2):
                            nc.tensor.matmul(qps[n],
                                             lhsT=wqS3[:, d, n * 128:(n + 1) * 128],
                                             rhs=xt, start=(d == 0), stop=(d == 8))
                    for n in range(2):
                        rope_evac(qps[n], qT[n], csS, snS, s, ev)
                    ups = [sm_tile(f"ups{i}") for i in range(4)]
                    for d in range(9):
                        xt = xqS3[:, d, s]
                        for i in range(4):
                            nc.tensor.matmul(ups[i][:, 0:256],
                                             lhsT=xt[:, i * 128:(i + 1) * 128],
                                             rhs=wuS3[:, d, :],
                                             start=(d == 0), stop=(d == 8))
                    # sigmoid(u) = 0.5*tanh(0.5*u) + 0.5
                    eu = ev.tile([128, 1024], f32, tag="eu", name="eu")
                    for i in range(4):
                        nc.scalar.activation(out=eu[:, i * 256:(i + 1) * 256],
                                             in_=ups[i][:, 0:256], func=TANH,
                                             scale=0.5)
                    nc.vector.tensor_scalar(sig[qc], eu, 0.5, 0.5,
                                            mybir.AluOpType.mult,
                                            mybir.AluOpType.add)

            # ---- KV: K^T (+RoPE) then V + gating, per q-chunk ----
            with tc.tile_pool(name="xvp", bufs=1) as xvpool:
                xvS = xvpool.tile([128, 9 * L], bf16, name="xvS")
                xvS3 = xvS.rearrange("p (c q) -> p c q", q=L)
                for qc in range(4):
                    s = slice(qc * 512, (qc + 1) * 512)
                    for d in range(9):
                        dma.dma_start(out=xvS3[:, d, s],
                                      in_=xv[d * 128:(d + 1) * 128, s])
                for qc in range(4):
                    s = slice(qc * 512, (qc + 1) * 512)
                    kps = [sm_tile(f"kps{n}") for n in range(2)]
                    for d in range(9):
                        xtk = xkS3[:, d, s]
                        for n in range(2):
                            nc.tensor.matmul(kps[n],
                                             lhsT=wkS3[:, d, n * 128:(n + 1) * 128],
                                             rhs=xtk, start=(d == 0), stop=(d == 8))
                    for n in range(2):
                        rope_evac(kps[n], kT[n], csS, snS, s, ev)
                    vps = [sm_tile(f"vps{i}") for i in range(4)]
                    for d in range(9):
                        xtv = xvS3[:, d, s]
                        for i in range(4):
                            nc.tensor.matmul(vps[i][:, 0:256],
                                             lhsT=xtv[:, i * 128:(i + 1) * 128],
                                             rhs=wvS3[:, d, :],
                                             start=(d == 0), stop=(d == 8))
                    for i in range(4):
                        kt_g = qc * 4 + i
                        vsrc = vps[i][:, 0:256].rearrange("p (h e) -> p h e", e=64)
                        ssrc = sig[qc][:, i * 256:(i + 1) * 256].rearrange(
                            "p (h e) -> p h e", e=64)
                        nc.vector.tensor_mul(vg4[:, kt_g, :, 0:64], vsrc, ssrc)

            # ---- Attention ----
            for h in range(4):
                n = h // 2
                r0 = (h % 2) * 64
                pvt = [ps.tile([65, 512], f32, tag="sm", bufs=4, name=f"pvt{qc}")
                       for qc in range(4)]
                for kt in range(16):
                    for hq in range(2):
                        st = ps.tile([128, 1024], f32, tag="st", bufs=2, name="st")
                        for s2 in range(2):
                            q0 = hq * 1024 + s2 * 512
                            nc.tensor.matmul(
                                st[:, s2 * 512:(s2 + 1) * 512],
                                lhsT=kT[n][r0:r0 + 64, kt * 128:(kt + 1) * 128],
                                rhs=qT[n][r0:r0 + 64, q0:q0 + 512],
                                start=True, stop=True)
                        if with_mask:
                            mt = mkpool.tile([128, 1024], f32, tag="mt", name="mt")
                            dma.dma_start(
                                out=mt,
                                in_=mk[kt * 128:(kt + 1) * 128,
                                       hq * 1024:(hq + 1) * 1024])
                            nc.vector.tensor_add(st, st, mt)
                        pt = ptpool.tile([128, 1024], bf16, tag="pt", name="pt")
                        nc.scalar.activation(out=pt, in_=st, func=EXP,
                                             scale=SCALE,
                                             bias=cbS[:, kt * 4 + h:kt * 4 + h + 1])
                        for s2 in range(2):
                            qc = hq * 2 + s2
                            nc.tensor.matmul(
                                pvt[qc],
                                lhsT=vg[:, kt * 260 + h * 65:kt * 260 + h * 65 + 65],
                                rhs=pt[:, s2 * 512:(s2 + 1) * 512],
                                start=(kt == 0), stop=(kt == 15))
                # evacuate numerators + denominators; batched reciprocal.
                rg = nrmpool.tile([128, 512], f32, tag="rg", name="rg")
                nc.gpsimd.memset(rg, 1.0)
                for qc in range(4):
                    nc.vector.tensor_copy(out=oT[n][r0:r0 + 64,
                                                    qc * 512:(qc + 1) * 512],
                                          in_=pvt[qc][0:64, :])
                    nc.vector.tensor_copy(out=rg[qc * 32:qc * 32 + 1, :],
                                          in_=pvt[qc][64:65, :])
                rinv = nrmpool.tile([128, 512], f32, tag="ri", name="rinv")
                nc.vector.reciprocal(out=rinv, in_=rg)
                drv = drmpool.tile([4, 512], f32, tag="drv", name="drv")
                dma.dma_start(out=drv,
                              in_=rinv.rearrange("(a b) f -> a b f", b=32)[:, 0, :])
                ib = nrmpool.tile([128, L], f32, tag="ib", bufs=2, name="ib")
                dma.dma_start(out=ib[r0:r0 + 64, :],
                              in_=drv.flatten()[:].partition_broadcast(64))
                for qc in range(4):
                    sl = slice(qc * 512, (qc + 1) * 512)
                    nc.vector.tensor_mul(oT[n][r0:r0 + 64, sl],
                                         oT[n][r0:r0 + 64, sl],
                                         ib[r0:r0 + 64, sl])

            # keep the PE clock warm across the normalize tail of head 3
            wt_ = ps.tile([128, 512], f32, tag="sm", bufs=4, name="warm")
            for j_ in range(20):
                nc.tensor.matmul(wt_, lhsT=woS[0][:, 0:128], rhs=woS[0][:, 0:512],
                                 start=(j_ == 0), stop=(j_ == 19))

            # ---- Out-projection ----
            with tc.tile_pool(name="otp", bufs=2) as otpool:
                for mt_i in range(8):
                    ot = otpool.tile([128, L], f32, tag="ot", name="ot")
                    ops = [sm_tile(f"op{qc}") for qc in range(4)]
                    for n2 in range(2):
                        for qc in range(4):
                            nc.tensor.matmul(
                                ops[qc],
                                lhsT=woS[n2][:, mt_i * 128:(mt_i + 1) * 128],
                                rhs=oT[n2][:, qc * 512:(qc + 1) * 512],
                                start=(n2 == 0), stop=(n2 == 1))
                    for qc in range(4):
                        nc.vector.tensor_copy(out=ot[:, qc * 512:(qc + 1) * 512],
                                              in_=ops[qc])
                    dma.dma_start(out=outT[mt_i * 128:(mt_i + 1) * 128, :], in_=ot)

    nc.finalize()
    return nc


def prep_inputs(query, key, value, attn_mask, action_ids, time_deltas,
                Wq, bq, Wk, bk, Wv, bv, Wu, bu, Wo, bo,
                action_emb, Wap, bap, td_emb, td_gate):
    """Host-side sharding: build the 8 per-core input maps."""
    query = np.asarray(query, np.float32)
    key = np.asarray(key, np.float32)
    value = np.asarray(value, np.float32)
    attn_mask = np.asarray(attn_mask)
    action_ids = np.asarray(action_ids)
    time_deltas = np.asarray(time_deltas)

    sig_gate = 1.0 / (1.0 + np.exp(-np.float64(td_gate)))
    with_mask = not bool(attn_mask.all())

    xq_b, xk_b, xv_b, cb_b, mk_b = [], [], [], [], []
    for b in range(B):
        ae = np.asarray(action_emb, np.float32)[action_ids[b]]      # [L, 16]
        xqa = np.zeros((DPAD, L), BF16)
        xqa[:D] = query[b].T.astype(BF16)
        xqa[D:D + 16] = ae.T.astype(BF16)
        xqa[D + 16] = BF16(1.0)
        xq_b.append(xqa)
        xka = np.zeros((DPAD, L), BF16)
        xka[:D] = key[b].T.astype(BF16)
        xka[D] = BF16(1.0)
        xk_b.append(xka)
        xva = np.zeros((DPAD, L), BF16)
        xva[:D] = value[b].T.astype(BF16)
        xva[D] = BF16(1.0)
        xv_b.append(xva)
        tdc = np.clip(time_deltas[b].astype(np.int64), 0, td_emb.shape[0] - 1)
        cb_b.append((sig_gate * np.asarray(td_emb, np.float32)[tdc]).astype(np.float32))
        if with_mask:
            m = np.where(attn_mask[b], np.float32(0.0), np.float32(-1e9))
            mk_b.append(np.ascontiguousarray(m.T))                  # [k, q]

    wq_a = np.zeros((DPAD, D), np.float32)
    wq_a[:D] = Wq
    wq_a[D + 16] = bq
    wu_a = np.zeros((DPAD, D), np.float32)
    wu_a[:D] = Wu
    wu_a[D:D + 16] = Wap
    wu_a[D + 16] = np.asarray(bu) + np.asarray(bap)
    wk_a = np.zeros((DPAD, D), np.float32)
    wk_a[:D] = Wk
    wk_a[D] = bk
    wv_a = np.zeros((DPAD, D), np.float32)
    wv_a[:D] = Wv
    wv_a[D] = bv

    # RoPE tables in [dh, pos] orientation, duplicated for the 2-head packing.
    # sin table carries the rotate_half sign: rows d<32 of each 64-block hold
    # -sin (they multiply q[d+32]), rows d>=32 hold +sin (multiply q[d-32]).
    inv_freq = 1.0 / (10000.0 ** (np.arange(0, DH, 2, dtype=np.float64) / DH))
    pos = np.arange(L, dtype=np.float64)
    freqs = pos[None, :] * inv_freq[:, None]            # [32, L]
    cos_t = np.repeat(np.cos(freqs), 2, axis=0)[:DH]    # [64, L]
    sin_t = np.repeat(np.sin(freqs), 2, axis=0)[:DH]
    ss_t = sin_t.copy()
    ss_t[0:32] = -ss_t[0:32]
    cs_t = np.ascontiguousarray(np.concatenate([cos_t, cos_t], 0), np.float32)
    sn_t = np.ascontiguousarray(np.concatenate([ss_t, ss_t], 0), np.float32)

    in_maps = []
    for c in range(NCORES):
        b, hg = c // 4, c % 4
        csl = slice(hg * NG, (hg + 1) * NG)
        cbc = cb_b[b][:, hg * 4:(hg + 1) * 4]                       # [L, 4]
        cbc = cbc.reshape(16, 128, 4).transpose(1, 0, 2).reshape(128, 64)
        m = {
            "xq": xq_b[b], "xk": xk_b[b], "xv": xv_b[b],
            "wq": wq_a[:, csl].astype(BF16), "wu": wu_a[:, csl].astype(BF16),
            "wk": wk_a[:, csl].astype(BF16), "wv": wv_a[:, csl].astype(BF16),
            "wo": np.asarray(Wo, np.float32)[csl, :].astype(BF16),
            "cb": np.ascontiguousarray(cbc, np.float32),
            "cs": cs_t, "sn": sn_t,
        }
        if with_mask:
            m["mk"] = mk_b[b]
        in_maps.append(m)
    return in_maps, with_mask


def gather_output(results, bo):
    """Sum head-group partials per batch, transpose, add bo."""
    out = np.empty((B, L, D), np.float32)
    for b in range(B):
        acc = results[b * 4]["outT"].astype(np.float32).copy()
        for g in range(1, 4):
            acc += results[b * 4 + g]["outT"]
        out[b] = acc.T + np.asarray(bo, np.float32)
    return out


def kernel(**inputs):
    from concourse.bass_utils import run_bass_kernel_spmd

    in_maps, with_mask = prep_inputs(**inputs)
    nc = build_bass(with_mask)
    res = run_bass_kernel_spmd(nc, in_maps, core_ids=list(range(NCORES)))
    return gather_output(res.results, inputs["bo"])

